# revision 1
# baseline (speedup 1.0000x reference)
"""Cox proportional-hazards negative partial log-likelihood, distributed
across 8 Trainium2 NeuronCores.

reference:
    risk_mask[i, j] = (time[j] >= time[i])
    risk_sum[i]     = sum_j exp(hazard[j]) * risk_mask[i, j]
    loss            = -mean((hazard - log(risk_sum)) * censor)

Strategy (rows i sharded 1024/core; every core sees the full j axis):
  * j is laid out as [128 partitions x 64 tiles], j = p*64 + t.
  * Per j-tile, a [128, 1024] mask tile m[p, i] = (t_i <= time_j) is produced
    on one of THREE engines (greedy load balance): VectorE / GpSimd
    (tensor_scalar is_le, exact compare) or ScalarE
    (sigmoid(LARGE*(time_j - t_i) + 2^16), which saturates to exactly 0/1
    after low-precision output rounding for every pair, ties and the
    diagonal included, since the minimum nonzero |time_j - t_i| of the fp32
    inputs is >= 2^-23 and LARGE*2^-23 = 2^17 dwarfs both the sigmoid's ~18
    saturation width and the 2^16 tie-shift).
  * TensorE reduces over the j partitions. In "fp8dr" mode masks are fp8
    ({0,1} exact) packed two j-tiles per DoubleRow matmul (0.5 cycles/row),
    with exp(hazard_j) as the stationary operand split into 3 fp8 terms
    (hi+lo+lo2, ~12 significant bits combined). In "bf16" mode masks are
    bf16 with bf16 hi+lo weights.
  * Each core returns its PSUM accumulator rows; the host sums the split
    rows, takes log, applies censor and the mean. Host work is O(N).
"""

import numpy as np

N = 8192
P = 128
NT = N // P          # 64 j-tiles
NPAIR = NT // 2      # 32 j-tile pairs (fp8 DoubleRow mode)
NCORES = 8
R = N // NCORES      # 1024 rows per core
RH = 512             # psum half (one fp32 bank)
LARGE = float(2.0**40)   # time-domain sigmoid scale (RANKS=False)
SHIFT = float(2.0**16)
RANKS = True             # compare int16 ranks (DVE 4x mode) instead of fp32 times
RLARGE = 64.0            # rank-domain sigmoid scale: |64*(rj-ri)+32| >= 32 saturates
RSHIFT = 32.0
ACT_MOD = 3          # t % ACT_MOD == ACT_PHASE tiles go to ScalarE
ACT_PHASE = 2
MODE = "fp8dr"       # "bf16" (hi/lo bf16 weights) or "fp8dr" (fp8 DoubleRow)
NWSPLIT = 3          # fp8 weight split terms (hi, lo, lo2)
MPAD = 4             # weight M padded so the pair-dim AP step is 16B-aligned

_CACHE: dict = {}


def _ensure_path():
    try:
        import concourse.bass  # noqa: F401
    except ImportError:
        import sys

        sys.path.insert(0, "/opt/trn_rl_repo")


def _build_program():
    import concourse.bass as bass
    import concourse.mybir as mybir
    from concourse import tile

    f32 = mybir.dt.float32
    bf16 = mybir.dt.bfloat16
    fp8 = mybir.dt.float8e4
    Alu = mybir.AluOpType
    Act = mybir.ActivationFunctionType

    out_rows = MPAD if MODE == "fp8dr" else 2

    nc = bass.Bass()
    time_all = nc.declare_dram_parameter("time_all", [N], f32, isOutput=False)
    hazard_all = nc.declare_dram_parameter("hazard_all", [N], f32, isOutput=False)
    i16 = mybir.dt.int16
    t_rows = nc.declare_dram_parameter("t_rows", [R], i16 if RANKS else f32, isOutput=False)
    risk2 = nc.declare_dram_parameter("risk2", [out_rows, R], f32, isOutput=True)

    # greedy 3-engine pair assignment by modeled per-pair cost (ns).
    # vec pairs run at DVE 4x (bf16 masks) but cost PE 864ns vs 214ns for
    # the fp8 DoubleRow pairs on act/pool — cap them to keep PE balanced.
    cost = {"vec": 1166.0, "act": 2000.0, "pool": 3200.0}
    load = {"vec": 3300.0, "act": 1100.0, "pool": 0.0}
    assign = []
    for q in range(NPAIR):
        eng = min(load, key=lambda e: load[e] + cost[e])
        load[eng] += cost[eng]
        assign.append(eng)

    with tile.TileContext(nc) as tc:
        n_act_t = sum(1 for t in range(NT) if t % ACT_MOD == ACT_PHASE)
        with (
            tc.tile_pool(name="const", bufs=1) as const,
            # one dedicated slot per mask tile: slot reuse would add WAR/WAW
            # semaphore waits, and walrus rejects >1 sync wait on ACT ops
            tc.tile_pool(
                name="vmask",
                bufs=assign.count("vec") if MODE == "fp8dr" else (NT - n_act_t),
            ) as vmaskp,
            tc.tile_pool(
                name="amask", bufs=assign.count("act") if MODE == "fp8dr" else n_act_t
            ) as amaskp,
            tc.tile_pool(name="pmask", bufs=max(assign.count("pool"), 1)) as pmaskp,
            tc.tile_pool(name="psum", bufs=1, space="PSUM") as psump,
        ):
            # ---- setup: load j-major data, [128, 64] with j = p*NT + t ----
            # t_bcast first: it gates every mask op and, as int16, is the
            # cheapest way to get the producers started earliest
            t_bcast = const.tile([P, R], i16 if RANKS else f32)
            nc.sync.dma_start(t_bcast[:], t_rows[None, :].to_broadcast((P, R)))
            time_sb = const.tile([P, NT], f32)
            nc.sync.dma_start(time_sb[:], time_all[:].rearrange("(p t) -> p t", t=NT))
            haz_sb = const.tile([P, NT], f32)
            nc.sync.dma_start(haz_sb[:], hazard_all[:].rearrange("(p t) -> p t", t=NT))

            exph = const.tile([P, NT], f32)
            nc.scalar.activation(exph[:], haz_sb[:], Act.Exp)

            # sigmoid bias: LARGE * time_j + 2^16 (exact in fp32 for time < 1)
            tscb = const.tile([P, NT], f32)
            if RANKS:
                nc.vector.tensor_scalar(tscb[:], time_sb[:], RLARGE, RSHIFT, Alu.mult, Alu.add)
            else:
                nc.vector.tensor_scalar(tscb[:], time_sb[:], LARGE, SHIFT, Alu.mult, Alu.add)

            if MODE == "fp8dr":
                # exp(hazard) as sum of NWSPLIT fp8 terms; residuals via fp32.
                # Casts run on ScalarE and subtractions on GpSimd so the mask
                # producers (mainly VectorE) start without setup serialization.
                splits8 = []
                resid = exph
                for s in range(NWSPLIT):
                    h8 = const.tile([P, NT], fp8, tag=f"h8_{s}")
                    nc.vector.tensor_copy(h8[:], resid[:])
                    splits8.append(h8)
                    if s < NWSPLIT - 1:
                        h32 = const.tile([P, NT], f32, tag=f"h32_{s}")
                        nc.vector.tensor_copy(h32[:], h8[:])
                        nresid = const.tile([P, NT], f32, tag=f"r32_{s}")
                        nc.vector.tensor_sub(nresid[:], resid[:], h32[:])
                        resid = nresid
                # stationary operand per pair q: w8[:, :, MPAD*q : MPAD*(q+1)]
                # = [128, 2, MPAD]; element (p, plane, part) = split_part of
                # exph at j-tile (2q+plane). Pair-dim AP step = MPAD*NPAIR
                # bytes (16B-aligned as the ISA requires).
                w8 = const.tile([P, 2, MPAD * NPAIR], fp8)
                nc.vector.memset(w8[:], 0.0)
                for s in range(NWSPLIT):
                    for plane in range(2):
                        nc.vector.tensor_copy(
                            w8[:, plane, s :: MPAD], splits8[s][:, plane::2]
                        )
            else:
                # exp(hazard) split into bf16 hi + lo, stored [P, 2, NT] so
                # wts[:, :, t] is a [128, 2] stationary operand.
                wts = const.tile([P, 2, NT], bf16)
                nc.vector.tensor_copy(wts[:, 0, :], exph[:])
                hi32 = const.tile([P, NT], f32)
                nc.vector.tensor_copy(hi32[:], wts[:, 0, :])
                lo32 = const.tile([P, NT], f32)
                nc.vector.tensor_sub(lo32[:], exph[:], hi32[:])
                nc.vector.tensor_copy(wts[:, 1, :], lo32[:])

            # prime each engine's vector clock so hot-loop instructions carry
            # at most one sync wait (walrus AC-struct limit)
            prime = const.tile([1, 4], f32)
            nc.scalar.activation(prime[:, 0:1], tscb[0:1, 0:1], Act.Copy)
            nc.scalar.activation(prime[:, 1:2], t_bcast[0:1, 0:1], Act.Copy)
            nc.vector.tensor_copy(prime[:, 2:3], t_bcast[0:1, 0:1])

            prisk = psump.tile([out_rows, R], f32)

            def emit_mask(out_ap, t, eng):
                if eng == "act":
                    nc.scalar.activation(
                        out_ap, t_bcast[:], Act.Sigmoid,
                        bias=tscb[:, t : t + 1], scale=-(RLARGE if RANKS else LARGE),
                    )
                else:
                    e = nc.vector if eng == "vec" else nc.gpsimd
                    e.tensor_scalar(
                        out_ap, t_bcast[:], time_sb[:, t : t + 1], None, Alu.is_le
                    )

            if MODE == "fp8dr":
                pools = {"vec": vmaskp, "act": amaskp, "pool": pmaskp}
                # hot loop: 2 mask planes + 2 DoubleRow matmuls per j-tile pair
                for q in range(NPAIR):
                    eng = assign[q]
                    mp = pools[eng].tile([P, 2, R], fp8, tag=f"{eng}mask")
                    for plane in range(2):
                        emit_mask(mp[:, plane, :], 2 * q + plane, eng)
                    for h in range(2):
                        nc.tensor.matmul(
                            prisk[:, h * RH : (h + 1) * RH],
                            w8[:, :, MPAD * q : MPAD * (q + 1)],
                            mp[:, :, h * RH : (h + 1) * RH],
                            start=(q == 0),
                            stop=(q == NPAIR - 1),
                            perf_mode=mybir.MatmulPerfMode.DoubleRow,
                        )
            else:
                # hot loop: one mask tile + 2 accumulating matmuls per j-tile
                for t in range(NT):
                    on_act = t % ACT_MOD == ACT_PHASE
                    m = (amaskp if on_act else vmaskp).tile(
                        [P, R], bf16, tag="amask" if on_act else "vmask"
                    )
                    emit_mask(m[:], t, "act" if on_act else "vec")
                    for h in range(2):
                        nc.tensor.matmul(
                            prisk[:, h * RH : (h + 1) * RH],
                            wts[:, :, t],
                            m[:, h * RH : (h + 1) * RH],
                            start=(t == 0),
                            stop=(t == NT - 1),
                        )

            # drain PSUM with DVE and ACT in parallel (one half each)
            out_sb = const.tile([out_rows, R], f32)
            nc.vector.tensor_copy(out_sb[:, 0:RH], prisk[:, 0:RH])
            nc.scalar.copy(out_sb[:, RH:R], prisk[:, RH:R])
            nc.sync.dma_start(risk2[:], out_sb[:])

    _split_sync_waits(nc, mybir)
    return nc


def _split_sync_waits(nc, mybir, max_waits=1):
    """walrus rejects instructions with too many sync waits (seen at 2 for
    ACT, 7 for the tile tail drain). Hoist excess waits onto same-engine
    NoOps inserted immediately before the offending instruction — waits
    execute in order on the engine sequencer, so this is equivalent."""
    serial = 0
    for f in nc.m.functions:
        for blk in f.blocks:
            il = blk.instructions
            pos = 0
            while pos < len(il):
                ins = il[pos]
                si = getattr(ins, "sync_info", None)
                if si is None or len(si.on_wait) <= max_waits:
                    pos += 1
                    continue
                waits = list(si.on_wait)
                ins.sync_info = mybir.SyncInfo(
                    on_wait=waits[-max_waits:], on_update=list(si.on_update)
                )
                for i in range(0, len(waits) - max_waits, max_waits):
                    nop = mybir.InstNoOp(name=f"I-waitsplit-{serial}", ins=[], outs=[])
                    serial += 1
                    nop.engine = ins.engine
                    nop.sync_info = mybir.SyncInfo(
                        on_wait=waits[i : i + max_waits], on_update=[]
                    )
                    nc.register_instruction(nop, overwrite=True)
                    il.insert(pos, nop)
                    pos += 1
                pos += 1


def _get_program():
    if "nc" not in _CACHE:
        _ensure_path()
        _CACHE["nc"] = _build_program()
    return _CACHE["nc"]


def kernel(hazard, time, censor):
    _ensure_path()
    from concourse.bass_utils import run_bass_kernel_spmd

    hazard = np.ascontiguousarray(np.asarray(hazard, dtype=np.float32))
    time = np.ascontiguousarray(np.asarray(time, dtype=np.float32))
    censor = np.asarray(censor, dtype=np.float32)
    if RANKS:
        # monotone relabeling: dense ranks with ties equal, so
        # (rank_j >= rank_i) <=> (time_j >= time_i) exactly
        _, rank = np.unique(time, return_inverse=True)
        key_f32 = np.ascontiguousarray(rank.astype(np.float32))
        key_i16 = np.ascontiguousarray(rank.astype(np.int16))
    else:
        key_f32 = time
        key_i16 = time

    nc = _get_program()
    in_maps = [
        {
            "time_all": key_f32,
            "hazard_all": hazard,
            "t_rows": key_i16[c * R : (c + 1) * R],
        }
        for c in range(NCORES)
    ]
    res = run_bass_kernel_spmd(nc, in_maps, list(range(NCORES)))
    risk = np.concatenate(
        [res.results[c]["risk2"].sum(axis=0, dtype=np.float64) for c in range(NCORES)]
    ).astype(np.float32)
    loss = -np.mean((hazard - np.log(risk)) * censor, dtype=np.float32)
    return np.float32(loss)



# revision 2
# speedup vs baseline: 3.8261x; 3.8261x over previous
"""Cox proportional-hazards negative partial log-likelihood on 8 Trainium2
NeuronCores.

reference:
    risk_mask[i, j] = (time[j] >= time[i])
    risk_sum[i]     = sum_j exp(hazard[j]) * risk_mask[i, j]
    loss            = -mean((hazard - log(risk_sum)) * censor)

Because the risk set {j : time_j >= time_i} is a prefix of the
descending-time order, the O(N^2) masked reduction collapses to a prefix
sum: with hazard sorted by time descending,

    S[k]        = sum_{k' <= k} exp(hazard_sorted[k'])
    risk_sum[i] = S[cnt_i - 1],   cnt_i = |{j : time_j >= time_i}|

which is exact under ties (every tie of time_i sits inside the prefix).

Split of work:
  * host: index bookkeeping only — argsort by time, searchsorted for
    cnt_i, final censored mean (the same role the previous kernel gave
    the host: rank relabeling via np.unique, 8-way gather, log, mean).
  * device (each core): all the FP math — exp(hazard) with a fused
    per-partition row-sum (ACT accum_out), a cross-partition carry via a
    [128x128] triangular matmul on PE, the 8192-long prefix scan via the
    DVE TensorTensorScanArith recurrence, and log of the sums.
    Data is laid out [128 partitions x 64], k = p*64 + t; the scan gives
    within-partition prefixes and the matmul supplies each partition's
    carry C[p] = sum of all full partitions p' < p as the scan's initial
    state.
  * sharding: per-core work is O(N) = 32KB streamed, far below the cost
    of any cross-core collective, so the scan is replicated on all 8
    cores (SPMD requires a single program; output-range sharding would
    need per-core programs) and core 0's output is used.
"""

import numpy as np

N = 8192
P = 128
NT = N // P          # 64 elements per partition
NCORES = 8

_CACHE: dict = {}


def _ensure_path():
    try:
        import concourse.bass  # noqa: F401
    except ImportError:
        import sys

        sys.path.insert(0, "/opt/trn_rl_repo")


def _build_program():
    import concourse.bass as bass
    import concourse.mybir as mybir
    from concourse import tile

    f32 = mybir.dt.float32
    Alu = mybir.AluOpType
    Act = mybir.ActivationFunctionType

    nc = bass.Bass()
    # hazard sorted by time descending, reshaped [128, 64] (k = p*64 + t)
    hs = nc.declare_dram_parameter("hs", [P, NT], f32, isOutput=False)
    # lmat[p, m] = 1.0 if p < m else 0.0 (strict prefix-carry matrix)
    lmat = nc.declare_dram_parameter("lmat", [P, P], f32, isOutput=False)
    # log of inclusive prefix sums of exp(hazard_sorted), same layout
    lgs = nc.declare_dram_parameter("lgs", [P, NT], f32, isOutput=True)

    with tile.TileContext(nc) as tc:
        with (
            tc.tile_pool(name="sb", bufs=1) as sb,
            tc.tile_pool(name="ps", bufs=1, space="PSUM") as psp,
        ):
            h = sb.tile([P, NT], f32)
            nc.sync.dma_start(h[:], hs[:])
            lm = sb.tile([P, P], f32)
            nc.sync.dma_start(lm[:], lmat[:])
            z = sb.tile([P, NT], f32)
            nc.vector.memset(z[:], 0.0)

            # e = exp(h); tot[p] = sum_t e[p, t] fused into the same ACT op
            e = sb.tile([P, NT], f32)
            tot = sb.tile([P, 1], f32)
            nc.scalar.activation(e[:], h[:], Act.Exp, accum_out=tot[:])

            # C[m] = sum_{p < m} tot[p]  (strict cross-partition carry)
            pc = psp.tile([P, 1], f32)
            nc.tensor.matmul(pc[:], lm[:], tot[:], start=True, stop=True)
            c = sb.tile([P, 1], f32)
            nc.vector.tensor_copy(c[:], pc[:])

            # s[p, t] = C[p] + sum_{t' <= t} e[p, t']
            s = sb.tile([P, NT], f32)
            nc.vector.tensor_tensor_scan(
                s[:], e[:], z[:], c[:, 0:1], Alu.add, Alu.add
            )

            out_sb = sb.tile([P, NT], f32)
            nc.scalar.activation(out_sb[:], s[:], Act.Ln)
            nc.sync.dma_start(lgs[:], out_sb[:])

    _split_sync_waits(nc, mybir)
    return nc


def _split_sync_waits(nc, mybir, max_waits=1):
    """walrus rejects instructions with too many sync waits. Hoist excess
    waits onto same-engine NoOps inserted immediately before the offending
    instruction — waits execute in order on the engine sequencer, so this
    is equivalent."""
    serial = 0
    for f in nc.m.functions:
        for blk in f.blocks:
            il = blk.instructions
            pos = 0
            while pos < len(il):
                ins = il[pos]
                si = getattr(ins, "sync_info", None)
                if si is None or len(si.on_wait) <= max_waits:
                    pos += 1
                    continue
                waits = list(si.on_wait)
                ins.sync_info = mybir.SyncInfo(
                    on_wait=waits[-max_waits:], on_update=list(si.on_update)
                )
                for i in range(0, len(waits) - max_waits, max_waits):
                    nop = mybir.InstNoOp(name=f"I-waitsplit-{serial}", ins=[], outs=[])
                    serial += 1
                    nop.engine = ins.engine
                    nop.sync_info = mybir.SyncInfo(
                        on_wait=waits[i : i + max_waits], on_update=[]
                    )
                    nc.register_instruction(nop, overwrite=True)
                    il.insert(pos, nop)
                    pos += 1
                pos += 1


def _get_program():
    if "nc" not in _CACHE:
        _ensure_path()
        _CACHE["nc"] = _build_program()
    return _CACHE["nc"]


def kernel(hazard, time, censor):
    _ensure_path()
    from concourse.bass_utils import run_bass_kernel_spmd

    hazard = np.asarray(hazard, dtype=np.float32)
    time = np.asarray(time, dtype=np.float32)
    censor = np.asarray(censor, dtype=np.float32)

    # descending-time order; ties may land in any order within their group
    pd = np.argsort(-time, kind="stable")
    hs2d = np.ascontiguousarray(hazard[pd].reshape(P, NT))
    lm = np.ascontiguousarray(np.triu(np.ones((P, P), dtype=np.float32), 1))

    nc = _get_program()
    in_maps = [{"hs": hs2d, "lmat": lm} for _ in range(NCORES)]
    res = run_bass_kernel_spmd(nc, in_maps, list(range(NCORES)))
    lgs = np.asarray(res.results[0]["lgs"], dtype=np.float32).reshape(N)

    # cnt_i = |{j : time_j >= time_i}|; risk_sum_i is the prefix at cnt_i-1
    asc = np.sort(time)
    cnt = N - np.searchsorted(asc, time, side="left")
    logrisk = lgs[cnt - 1]
    loss = -np.mean((hazard - logrisk) * censor, dtype=np.float32)
    return np.float32(loss)


# revision 4
# speedup vs baseline: 4.0097x; 1.0480x over previous
"""Cox proportional-hazards negative partial log-likelihood on 8 Trainium2
NeuronCores.

reference:
    risk_mask[i, j] = (time[j] >= time[i])
    risk_sum[i]     = sum_j exp(hazard[j]) * risk_mask[i, j]
    loss            = -mean((hazard - log(risk_sum)) * censor)

Because the risk set {j : time_j >= time_i} is a prefix of the
descending-time order, the O(N^2) masked reduction collapses to a prefix
sum: with hazard sorted by time descending,

    S[k]        = sum_{k' <= k} exp(hazard_sorted[k'])
    risk_sum[i] = S[cnt_i - 1],   cnt_i = |{j : time_j >= time_i}|

which is exact under ties (every tie of time_i sits inside the prefix).

Split of work:
  * host: index bookkeeping only — argsort by time, searchsorted for
    cnt_i, final censored mean (the same role the previous kernel gave
    the host: rank relabeling via np.unique, 8-way gather, log, mean).
  * device (each core): all the FP math — exp(hazard) with a fused
    per-partition row-sum (ACT accum_out), a cross-partition carry via a
    [128x128] triangular matmul on PE, the 8192-long prefix scan via the
    DVE TensorTensorScanArith recurrence, and log of the sums.
    Data is laid out [128 partitions x 64], k = p*64 + t; the scan gives
    within-partition prefixes and the matmul supplies each partition's
    carry C[p] = sum of all full partitions p' < p as the scan's initial
    state.
  * sharding: per-core work is O(N) = 32KB streamed, far below the cost
    of any cross-core collective, so the scan is replicated on all 8
    cores (SPMD requires a single program; output-range sharding would
    need per-core programs) and core 0's output is used.
"""

import numpy as np

N = 8192
P = 128
NT = N // P          # 64 elements per partition
NCORES = 8

_CACHE: dict = {}


def _ensure_path():
    try:
        import concourse.bass  # noqa: F401
    except ImportError:
        import sys

        sys.path.insert(0, "/opt/trn_rl_repo")


def _build_program():
    import concourse.bass as bass
    import concourse.mybir as mybir
    from concourse import tile

    f32 = mybir.dt.float32
    Alu = mybir.AluOpType
    Act = mybir.ActivationFunctionType

    i32 = mybir.dt.int32

    nc = bass.Bass()
    # hazard sorted by time descending, reshaped [128, 64] (k = p*64 + t)
    hs = nc.declare_dram_parameter("hs", [P, NT], f32, isOutput=False)
    # log of inclusive prefix sums of exp(hazard_sorted), same layout
    lgs = nc.declare_dram_parameter("lgs", [P, NT], f32, isOutput=True)

    with tile.TileContext(nc) as tc:
        with (
            tc.tile_pool(name="sb", bufs=1) as sb,
            tc.tile_pool(name="ps", bufs=1, space="PSUM") as psp,
        ):
            h = sb.tile([P, NT], f32)
            nc.sync.dma_start(h[:], hs[:])
            z = sb.tile([P, NT], f32)
            nc.vector.memset(z[:], 0.0)

            # lm[p, m] = (p < m), built on the otherwise-idle GpSimd engine
            # while the input DMA is in flight
            ii = sb.tile([P, P], i32)
            nc.gpsimd.iota(ii[:], [[1, P]], base=0, channel_multiplier=-1)
            lm = sb.tile([P, P], f32)
            nc.gpsimd.tensor_scalar(lm[:], ii[:], 0.0, None, Alu.is_gt)

            # e = exp(h); tot[p] = sum_t e[p, t] fused into the same ACT op
            e = sb.tile([P, NT], f32)
            tot = sb.tile([P, 1], f32)
            nc.scalar.activation(e[:], h[:], Act.Exp, accum_out=tot[:])

            # s[p, t] = sum_{t' <= t} e[p, t']  (runs on DVE concurrently
            # with the ACT accumulator read that produces tot)
            s = sb.tile([P, NT], f32)
            nc.vector.tensor_tensor_scan(
                s[:], e[:], z[:], 0.0, Alu.add, Alu.add
            )

            # C[m] = sum_{p < m} tot[p]  (strict cross-partition carry)
            pc = psp.tile([P, 1], f32)
            nc.tensor.matmul(pc[:], lm[:], tot[:], start=True, stop=True)
            c = sb.tile([P, 1], f32)
            nc.vector.tensor_copy(c[:], pc[:])

            # lgs = Ln(s + C[p]) — the carry add rides the activation bias
            out_sb = sb.tile([P, NT], f32)
            nc.scalar.activation(out_sb[:], s[:], Act.Ln, bias=c[:, 0:1])
            nc.sync.dma_start(lgs[:], out_sb[:])

    _split_sync_waits(nc, mybir)
    return nc


def _split_sync_waits(nc, mybir, max_waits=1):
    """walrus rejects instructions with too many sync waits. Hoist excess
    waits onto same-engine NoOps inserted immediately before the offending
    instruction — waits execute in order on the engine sequencer, so this
    is equivalent."""
    serial = 0
    for f in nc.m.functions:
        for blk in f.blocks:
            il = blk.instructions
            pos = 0
            while pos < len(il):
                ins = il[pos]
                si = getattr(ins, "sync_info", None)
                if si is None or len(si.on_wait) <= max_waits:
                    pos += 1
                    continue
                waits = list(si.on_wait)
                ins.sync_info = mybir.SyncInfo(
                    on_wait=waits[-max_waits:], on_update=list(si.on_update)
                )
                for i in range(0, len(waits) - max_waits, max_waits):
                    nop = mybir.InstNoOp(name=f"I-waitsplit-{serial}", ins=[], outs=[])
                    serial += 1
                    nop.engine = ins.engine
                    nop.sync_info = mybir.SyncInfo(
                        on_wait=waits[i : i + max_waits], on_update=[]
                    )
                    nc.register_instruction(nop, overwrite=True)
                    il.insert(pos, nop)
                    pos += 1
                pos += 1


def _get_program():
    if "nc" not in _CACHE:
        _ensure_path()
        _CACHE["nc"] = _build_program()
    return _CACHE["nc"]


def kernel(hazard, time, censor):
    _ensure_path()
    from concourse.bass_utils import run_bass_kernel_spmd

    hazard = np.asarray(hazard, dtype=np.float32)
    time = np.asarray(time, dtype=np.float32)
    censor = np.asarray(censor, dtype=np.float32)

    # descending-time order; ties may land in any order within their group
    pd = np.argsort(-time, kind="stable")
    hs2d = np.ascontiguousarray(hazard[pd].reshape(P, NT))

    nc = _get_program()
    in_maps = [{"hs": hs2d} for _ in range(NCORES)]
    res = run_bass_kernel_spmd(nc, in_maps, list(range(NCORES)))
    lgs = np.asarray(res.results[0]["lgs"], dtype=np.float32).reshape(N)

    # cnt_i = |{j : time_j >= time_i}|; risk_sum_i is the prefix at cnt_i-1
    asc = np.sort(time)
    cnt = N - np.searchsorted(asc, time, side="left")
    logrisk = lgs[cnt - 1]
    loss = -np.mean((hazard - logrisk) * censor, dtype=np.float32)
    return np.float32(loss)
